# revision 40
# baseline (speedup 1.0000x reference)
"""Multi-head attention (B=4, S=2048, D=1024, H=16, dk=dv=64) on 8 TRN2 cores.

Sharding: core = (batch b, head-group g): data-parallel over batch (4) x
tensor-parallel over heads (2 groups of 8). Each core computes its batch's
Q/K/V projections for its 8 heads, attention, and a partial output
projection over its heads' rows of Wo. The host sums the two partial
outputs per batch.

Per-core kernel, all matmul operands bf16 (fp32 PSUM accumulate):
  xT is DMA'd once into SBUF (bf16, 32KB/partition) and reused by all
  projections. Heads are processed in 4 pairs; per pair Q/K are projected
  pair-packed ([2x64 dk, S]).

  Attention is a flat pipeline over (pair, query-block, key-chunk) steps
  paced by the ACT engine (exp of the 128x1024 score tile is the global
  floor at ~266us/core). Per key chunk both heads' scores land in one
  [128, 1024] PSUM tile and one exp covers both. The AV matmul is
  query-stationary: lhsT = exp-tile slice [128 s, 128 q], rhs =
  [V_h | ones] [128 s, 65] -> PSUM [128 q, 65] accumulated over 16 key
  chunks; column 64 is the softmax denominator (65 rows/matmul instead of
  512 for the value-stationary form). Only the first of the four
  query-chunk groups per PSUM bank passes start=True: start zeroes the
  whole 2KB bank, so the other groups bootstrap off the bank-wide
  pending-zero. After a block's 16 chunks the accumulator is copied to
  SBUF (freeing the bank), normalized with a DVE reciprocal +
  per-partition-scalar multiply, and PE-transposed back to [hv, q] for
  the output projection.

  All projection / V / output-projection matmul groups are emitted at a
  strongly deprioritized Tile priority: the list scheduler then slots
  them into PE gaps behind the attention stream instead of ahead of it,
  which keeps the ACT queue fed.
"""

from contextlib import nullcontext

import numpy as np

import concourse.bacc as bacc
import concourse.tile as tile
import concourse.mybir as mybir
from concourse.bass_utils import run_bass_kernel_spmd

F32 = mybir.dt.float32
BF16 = mybir.dt.bfloat16
EXP = mybir.ActivationFunctionType.Exp

P = 128
S = 2048
D = 1024
DK = 64
HPC = 8            # heads per core
NSC = S // P       # 16 key chunks of 128
NST = 4            # s-tiles of 512
STW = S // NST     # 512
NDC = D // P       # 8 d_model chunks
NPAIR = HPC // 2   # 4 head pairs
NQB = S // STW     # 4 query blocks of 512
NQC = STW // P     # 4 query chunks of 128 per block
SCALE = 1.0 / np.sqrt(DK)
LOWP = 10**7       # priority offset pushing filler behind the attention stream


def build_kernel():
    nc = bacc.Bacc("TRN2", target_bir_lowering=False, debug=False)

    xt_d = nc.dram_tensor("xt", [D, S], BF16, kind="ExternalInput")
    wq_d = nc.dram_tensor("wq", [NPAIR, P, NDC * P], BF16, kind="ExternalInput")
    wk_d = nc.dram_tensor("wk", [NPAIR, P, NDC * P], BF16, kind="ExternalInput")
    wv_d = nc.dram_tensor("wv", [D, HPC * DK], BF16, kind="ExternalInput")
    wo_d = nc.dram_tensor("wo", [HPC * DK, D], BF16, kind="ExternalInput")
    id_d = nc.dram_tensor("ident", [P, P], BF16, kind="ExternalInput")
    out_d = nc.dram_tensor("out", [S, D], F32, kind="ExternalOutput")

    xt_ap = xt_d.ap().rearrange("(dc p) s -> p dc s", p=P)

    with tile.TileContext(nc) as tc:
        with tc.tile_pool(name="persist", bufs=1) as persist, \
             tc.tile_pool(name="qkpool", bufs=3) as qkpool, \
             tc.tile_pool(name="wqkp", bufs=3) as wqkp, \
             tc.tile_pool(name="expp", bufs=8) as expp, \
             tc.tile_pool(name="avsbp", bufs=3) as avsbp, \
             tc.tile_pool(name="htsp", bufs=4) as htsp, \
             tc.tile_pool(name="recp", bufs=8) as recp, \
             tc.tile_pool(name="stage", bufs=8) as stage, \
             tc.tile_pool(name="scps", bufs=2, space="PSUM") as scps, \
             tc.tile_pool(name="avps", bufs=1, space="PSUM") as avps, \
             tc.tile_pool(name="qkps", bufs=2, space="PSUM") as qkps:

            xts = persist.tile([P, NDC, S], BF16, tag="xts")
            v520 = persist.tile([P, NSC, HPC, DK + 1], BF16, tag="v520")
            hn = persist.tile([P, NPAIR, S], BF16, tag="hn")
            wo_sb = persist.tile([P, NPAIR, D], BF16, tag="wo")
            wv_sb = persist.tile([P, NDC, HPC * DK], BF16, tag="wv")
            id_sb = persist.tile([P, P], BF16, tag="id")

            # ---------------- prologue DMAs ----------------
            wqk_tiles = {}

            def fetch_wqk(pr):
                wqp = wqkp.tile([P, NDC, P], BF16, tag="wqp", name=f"wqp{pr}")
                wkp = wqkp.tile([P, NDC, P], BF16, tag="wkp", name=f"wkp{pr}")
                nc.sync.dma_start(
                    wqp[:], wq_d.ap()[pr].rearrange("p (dc c) -> p dc c", c=P))
                nc.sync.dma_start(
                    wkp[:], wk_d.ap()[pr].rearrange("p (dc c) -> p dc c", c=P))
                wqk_tiles[pr] = (wqp, wkp)

            fetch_wqk(0)
            # xT in 16 chunks; query-block-0 columns first so pair-0 can
            # start; wv before the later chunks (V projection rides early),
            # ident/wo last (needed only from the first norm / pair 3).
            def xts_qtr(qtr):
                for dh in range(NDC // 2):
                    nc.sync.dma_start(
                        xts[:, 2 * dh:2 * dh + 2, qtr * 512:(qtr + 1) * 512],
                        xt_ap[:, 2 * dh:2 * dh + 2, qtr * 512:(qtr + 1) * 512])

            xts_qtr(0)
            xts_qtr(1)
            nc.sync.dma_start(
                wv_sb[:], wv_d.ap().rearrange("(dc p) c -> p dc c", p=P))
            for qtr in range(2, 4):
                xts_qtr(qtr)
            nc.sync.dma_start(id_sb[:], id_d.ap())
            nc.sync.dma_start(
                wo_sb[:], wo_d.ap().rearrange("(ci p) d -> p ci d", p=P))
            nc.vector.memset(v520[:, :, :, DK:DK + 1], 1.0)

            qk_tiles = {0: (qkpool.tile([P, S], BF16, tag="qt", name="qt0"),
                            qkpool.tile([P, S], BF16, tag="kt", name="kt0"))}

            # ~3.4us of dummy matmuls on a zero scratch tile: the PE is
            # waiting on the prologue DMAs anyway, and this lifts it out of
            # the mid p-state before the first real projection group.
            scratch = persist.tile([P, STW], BF16, tag="scratch")
            nc.vector.memset(scratch[:], 0.0)
            for i in range(8):
                wps = qkps.tile([P, STW], F32, tag="qkp", name=f"warm{i}")
                nc.tensor.matmul(wps[:], scratch[:, 0:P], scratch[:],
                                 start=True, stop=True)

            # ---------------- helpers ----------------
            def lp(on):
                return tc.high_priority(offset=-LOWP) if on else nullcontext()

            def qk_group(pr, which, st, sub_sc=False, lowp=False):
                """Project qtp/ktp columns st*512:(st+1)*512 for pair pr."""
                w_sb = wqk_tiles[pr][0 if which == "q" else 1]
                dst = qk_tiles[pr][0 if which == "q" else 1]
                ps = qkps.tile([P, STW], F32, tag="qkp", name=f"{which}{pr}{st}")
                if sub_sc:
                    # key-chunk granular psum + copies (fast prologue start)
                    for scl in range(4):
                        for dc in range(NDC):
                            nc.tensor.matmul(
                                ps[:, scl * P:(scl + 1) * P], w_sb[:, dc, :],
                                xts[:, dc, (st * 4 + scl) * P:(st * 4 + scl + 1) * P],
                                start=(dc == 0), stop=(dc == NDC - 1))
                        nc.vector.tensor_copy(
                            dst[:, (st * 4 + scl) * P:(st * 4 + scl + 1) * P],
                            ps[:, scl * P:(scl + 1) * P])
                else:
                    with lp(lowp):
                        for dc in range(NDC):
                            nc.tensor.matmul(
                                ps[:], w_sb[:, dc, :],
                                xts[:, dc, st * STW:(st + 1) * STW],
                                start=(dc == 0), stop=(dc == NDC - 1))
                        nc.vector.tensor_copy(
                            dst[:, st * STW:(st + 1) * STW], ps[:])

            def v_group(pr, sc, lowp=False):
                """Project V for pair pr's two heads, key chunk sc."""
                ps = qkps.tile([P, STW], F32, tag="qkp", name=f"v{pr}{sc}")
                with lp(lowp):
                    for dc in range(NDC):
                        nc.tensor.matmul(
                            ps[:, 0:P], xts[:, dc, sc * P:(sc + 1) * P],
                            wv_sb[:, dc, pr * P:(pr + 1) * P],
                            start=(dc == 0), stop=(dc == NDC - 1))
                    nc.vector.tensor_copy(
                    v520[:, sc, 2 * pr:2 * pr + 2, 0:DK],
                    ps[:, 0:P].rearrange("p (h v) -> p h v", v=DK))

            def out_group(sc_out, dmh, pool=None, lowp=False):
                if pool is scps:
                    pa = pool.tile([P, 2 * STW], F32, tag="scp",
                                   name=f"o{sc_out}{dmh}")[:, 0:STW]
                else:
                    pa = qkps.tile([P, STW], F32, tag="qkp",
                                   name=f"o{sc_out}{dmh}")[:]
                with lp(lowp):
                    for ci in range(NPAIR):
                        nc.tensor.matmul(
                            pa, hn[:, ci, sc_out * P:(sc_out + 1) * P],
                            wo_sb[:, ci, dmh * 512:(dmh + 1) * 512],
                            start=(ci == 0), stop=(ci == NPAIR - 1))
                    osb = stage.tile([P, STW], F32, tag="ostage", name="osb")
                    nc.vector.tensor_copy(osb[:], pa)
                    nc.sync.dma_start(
                        out_d.ap()[sc_out * P:(sc_out + 1) * P,
                                   dmh * 512:(dmh + 1) * 512],
                        osb[:])

            def scores_exp(pr, qb, sc):
                qtp, ktp = qk_tiles[pr]
                q0 = qb * STW
                scp = scps.tile([P, 2 * STW], F32, tag="scp", name="scp")
                for j in range(2):
                    nc.tensor.matmul(
                        scp[:, j * STW:(j + 1) * STW],
                        ktp[j * DK:(j + 1) * DK, sc * P:(sc + 1) * P],
                        qtp[j * DK:(j + 1) * DK, q0:q0 + STW],
                        start=True, stop=True)
                ex = expp.tile([P, 2 * STW], BF16, tag="ex", name="ex")
                nc.scalar.activation(ex[:], scp[:], EXP, scale=float(SCALE))
                return ex

            av_cur = [None]

            def av_mms(pr, qb, sc, ex):
                if sc == 0:
                    av_cur[0] = [avps.tile([P, NQC * 65], F32, tag=f"av{j}",
                                           name=f"av{j}_{pr}{qb}")
                                 for j in range(2)]
                av = av_cur[0]
                for j in range(2):
                    for qc in range(NQC):
                        nc.tensor.matmul(
                            av[j][:, qc * 65:(qc + 1) * 65],
                            ex[:, j * STW + qc * P:j * STW + (qc + 1) * P],
                            v520[:, sc, 2 * pr + j, :],
                            # start=True zeroes the whole 2KB psum bank, so
                            # only the first of the 4 interleaved qc groups
                            # starts; the rest overwrite-on-first-write via
                            # the bank-wide pending-zero.
                            start=(sc == 0 and qc == 0),
                            stop=(sc == NSC - 1),
                            skip_group_check=True)
                if sc == NSC - 1:
                    if (pr, qb) == (NPAIR - 1, NQB - 1):
                        return av   # epilogue: no next block, read PSUM
                    avsb = avsbp.tile([P, 2, NQC * 65], F32, tag="avsb",
                                      name=f"avsb{pr}{qb}")
                    for j in range(2):
                        nc.vector.tensor_copy(avsb[:, j, :], av[j][:])
                    return avsb
                return None

            def norm_qc(pr, qb, avsb, qc):
                def sl(j, lo, hi):
                    if isinstance(avsb, list):
                        return avsb[j][:, lo:hi]
                    return avsb[:, j, lo:hi]

                """Normalize + transpose one 128-query chunk of both heads.

                The transpose lands in the (between-blocks idle) av0 PSUM
                bank rather than the qkp pool, keeping the qkp rotation
                free for projection / output-projection groups. Emission
                order guarantees the next block's AV matmuls (which re-start
                the bank) come after the hn copies below.
                """
                hts = htsp.tile([P, P], BF16, tag="hts", name=f"h{pr}{qb}{qc}")
                for j in range(2):
                    rec = recp.tile([P, 1], F32, tag="rec", name="rec")
                    nc.vector.reciprocal(
                        rec[:], sl(j, qc * 65 + DK, qc * 65 + DK + 1))
                    nc.vector.tensor_scalar_mul(
                        hts[:, j * DK:(j + 1) * DK],
                        sl(j, qc * 65, qc * 65 + DK), rec[:])
                # the transpose must own its whole PSUM bank: start=True
                # pends the full 2KB zero-region, which silently clobbers
                # any other accumulation group sharing the bank
                tp = qkps.tile([P, STW], F32, tag="qkp", name=f"t{pr}{qb}{qc}")
                tpb = tp[:, 0:P // 2].bitcast(BF16)
                nc.tensor.transpose(tpb, hts[:], id_sb[:])
                nc.vector.tensor_copy(
                    hn[:, pr, (qb * NQC + qc) * P:(qb * NQC + qc + 1) * P],
                    tpb)

            # ---------------- prologue compute (pair 0 heads) --------------
            qk_group(0, "q", 0, sub_sc=True)
            qk_group(0, "k", 0, sub_sc=True)
            for sc in range(4):
                v_group(0, sc)
            for st in range(1, NST):
                qk_group(0, "q", st, lowp=True)
                qk_group(0, "k", st, lowp=True)
            for sc in range(4, NSC):
                v_group(0, sc, lowp=True)

            # ---------------- flat attention pipeline ----------------
            steps = [(pr, qb, sc)
                     for pr in range(NPAIR)
                     for qb in range(NQB)
                     for sc in range(NSC)]

            # Projection filler is released in per-query-block drops sized
            # to each block's PE slack (the scheduler consumes low-priority
            # work eagerly, so one big drop per pair starves the later
            # blocks of that pair). Each drop lands one block before its
            # deadline; pair n's tiles/weights are set up in pair n-1.
            import collections as _c
            drops = _c.defaultdict(list)
            MID = NSC // 2

            def alloc_pair(n):
                def _do():
                    qk_tiles[n] = (
                        qkpool.tile([P, S], BF16, tag="qt", name=f"qt{n}"),
                        qkpool.tile([P, S], BF16, tag="kt", name=f"kt{n}"))
                    fetch_wqk(n)
                return _do

            def qd(n, which, st):
                return lambda: qk_group(n, which, st, lowp=True)

            def vd(n, sc):
                return lambda: v_group(n, sc, lowp=True)

            for n in range(1, NPAIR):
                drops[(n - 1, 1, 0)] += [alloc_pair(n), qd(n, "q", 0)]
                drops[(n - 1, 1, MID)] += [qd(n, "k", 0)]
                drops[(n - 1, 2, 0)] += [qd(n, "q", 1)]
                drops[(n - 1, 2, MID)] += [qd(n, "k", 1)]
                drops[(n - 1, 3, 0)] += [vd(n, sc) for sc in range(4)]
                drops[(n - 1, 3, MID)] += [vd(n, sc) for sc in range(4, 8)]
                drops[(n, 0, 0)] += [qd(n, "k", 2), qd(n, "k", 3)]
                drops[(n, 0, 0)] += [vd(n, sc) for sc in range(8, 12)]
                drops[(n, 0, MID)] += [vd(n, sc) for sc in range(12, NSC)]
                # late Q blocks: inside pair n for n<3, a block earlier for
                # pair 3 whose own span carries the output projections
                if n < NPAIR - 1:
                    drops[(n, 1, 0)] += [qd(n, "q", 2)]
                    drops[(n, 1, MID)] += [qd(n, "q", 3)]
                else:
                    drops[(n - 1, 3, 0)] += [qd(n, "q", 2)]
                    drops[(n - 1, 3, MID)] += [qd(n, "q", 3)]

            exs = {}
            for t in range(len(steps) + 1):
                if t < len(steps):
                    cur = steps[t]
                    if cur[2] in (0, MID):
                        for item in drops.pop(cur, ()):
                            item()
                    exs[cur] = scores_exp(*cur)
                if t > 0:
                    prv = steps[t - 1]
                    avsb = av_mms(*prv, exs.pop(prv))
                    if avsb is not None:
                        ppr, pqb = prv[0], prv[1]
                        for qc in range(NQC):
                            norm_qc(ppr, pqb, avsb, qc)
                            if ppr == NPAIR - 1:
                                so = pqb * NQC + qc
                                if pqb < NQB - 1:
                                    # half now, half at this block's midpoint
                                    # so the scheduler can't burn it all in
                                    # the front of the block
                                    out_group(so, 0, lowp=True)
                                    drops[(ppr, pqb + 1, MID)].append(
                                        lambda so=so: out_group(
                                            so, 1, lowp=True))
                                else:
                                    # epilogue: scores are done, so borrow
                                    # the scp banks for a 4-deep rotation
                                    out_group(so, 0, pool=scps)
                                    out_group(so, 1)

    nc.compile()
    return nc


_NC_CACHE = None


def _get_nc():
    global _NC_CACHE
    if _NC_CACHE is None:
        _NC_CACHE = build_kernel()
    return _NC_CACHE


def kernel(x, Wq, Wk, Wv, Wo):
    import ml_dtypes
    bf16 = ml_dtypes.bfloat16

    x = np.asarray(x, dtype=np.float32)
    Wq = np.asarray(Wq, dtype=np.float32)
    Wk = np.asarray(Wk, dtype=np.float32)
    Wv = np.asarray(Wv, dtype=np.float32)
    Wo = np.asarray(Wo, dtype=np.float32)
    B = x.shape[0]
    ident = np.eye(P, dtype=bf16)

    in_maps = []
    for core in range(8):
        b, g = divmod(core, 2)
        hs = g * HPC
        xt = np.ascontiguousarray(x[b].T).astype(bf16)
        def pack_w(W):
            pairs = []
            for p in range(NPAIR):
                m = np.concatenate([W[hs + 2 * p], W[hs + 2 * p + 1]], axis=1)
                # [D, 128] -> [128 part, NDC*128]: per-partition contiguous
                # so the weight DMA is one 2KB run per partition
                m = m.reshape(NDC, P, P).transpose(1, 0, 2).reshape(P, NDC * P)
                pairs.append(m)
            return np.stack(pairs).astype(bf16)

        wq = pack_w(Wq)
        wk = pack_w(Wk)
        wv = np.concatenate([Wv[hs + h] for h in range(HPC)], axis=1).astype(bf16)
        wo = np.ascontiguousarray(Wo[hs * DK:(hs + HPC) * DK, :]).astype(bf16)
        in_maps.append({"xt": xt, "wq": wq, "wk": wk, "wv": wv, "wo": wo,
                        "ident": ident})

    nc = _get_nc()
    res = run_bass_kernel_spmd(nc, in_maps, core_ids=list(range(8))).results

    out = np.empty((B, S, D), dtype=np.float32)
    for b in range(B):
        out[b] = res[2 * b]["out"] + res[2 * b + 1]["out"]
    return out


# revision 41
# speedup vs baseline: 1.0029x; 1.0029x over previous
"""Multi-head attention (B=4, S=2048, D=1024, H=16, dk=dv=64) on 8 TRN2 cores.

Sharding: core = (batch b, head-group g): data-parallel over batch (4) x
tensor-parallel over heads (2 groups of 8). Each core computes its batch's
Q/K/V projections for its 8 heads, attention, and a partial output
projection over its heads' rows of Wo. The host sums the two partial
outputs per batch.

Per-core kernel, all matmul operands bf16 (fp32 PSUM accumulate):
  xT is DMA'd once into SBUF (bf16, 32KB/partition) and reused by all
  projections. Heads are processed in 4 pairs; per pair Q/K are projected
  pair-packed ([2x64 dk, S]).

  Attention is a flat pipeline over (pair, query-block, key-chunk) steps
  paced by the ACT engine (exp of the 128x1024 score tile is the global
  floor at ~266us/core). Per key chunk both heads' scores land in one
  [128, 1024] PSUM tile and one exp covers both. The AV matmul is
  query-stationary: lhsT = exp-tile slice [128 s, 128 q], rhs =
  [V_h | ones] [128 s, 65] -> PSUM [128 q, 65] accumulated over 16 key
  chunks; column 64 is the softmax denominator (65 rows/matmul instead of
  512 for the value-stationary form). Only the first of the four
  query-chunk groups per PSUM bank passes start=True: start zeroes the
  whole 2KB bank, so the other groups bootstrap off the bank-wide
  pending-zero. After a block's 16 chunks the accumulator is copied to
  SBUF (freeing the bank), normalized with a DVE reciprocal +
  per-partition-scalar multiply, and PE-transposed back to [hv, q] for
  the output projection.

  All projection / V / output-projection matmul groups are emitted at a
  strongly deprioritized Tile priority: the list scheduler then slots
  them into PE gaps behind the attention stream instead of ahead of it,
  which keeps the ACT queue fed.
"""

from contextlib import nullcontext

import numpy as np

import concourse.bacc as bacc
import concourse.tile as tile
import concourse.mybir as mybir
from concourse.bass_utils import run_bass_kernel_spmd

F32 = mybir.dt.float32
BF16 = mybir.dt.bfloat16
EXP = mybir.ActivationFunctionType.Exp

P = 128
S = 2048
D = 1024
DK = 64
HPC = 8            # heads per core
NSC = S // P       # 16 key chunks of 128
NST = 4            # s-tiles of 512
STW = S // NST     # 512
NDC = D // P       # 8 d_model chunks
NPAIR = HPC // 2   # 4 head pairs
NQB = S // STW     # 4 query blocks of 512
NQC = STW // P     # 4 query chunks of 128 per block
SCALE = 1.0 / np.sqrt(DK)
LOWP = 10**7       # priority offset pushing filler behind the attention stream


def build_kernel():
    nc = bacc.Bacc("TRN2", target_bir_lowering=False, debug=False)

    xt_d = nc.dram_tensor("xt", [D, S], BF16, kind="ExternalInput")
    wq_d = nc.dram_tensor("wq", [NPAIR, P, NDC * P], BF16, kind="ExternalInput")
    wk_d = nc.dram_tensor("wk", [NPAIR, P, NDC * P], BF16, kind="ExternalInput")
    wv_d = nc.dram_tensor("wv", [D, HPC * DK], BF16, kind="ExternalInput")
    wo_d = nc.dram_tensor("wo", [HPC * DK, D], BF16, kind="ExternalInput")
    id_d = nc.dram_tensor("ident", [P, P], BF16, kind="ExternalInput")
    out_d = nc.dram_tensor("out", [S, D], F32, kind="ExternalOutput")

    xt_ap = xt_d.ap().rearrange("(dc p) s -> p dc s", p=P)

    with tile.TileContext(nc) as tc:
        with tc.tile_pool(name="persist", bufs=1) as persist, \
             tc.tile_pool(name="qkpool", bufs=3) as qkpool, \
             tc.tile_pool(name="wqkp", bufs=3) as wqkp, \
             tc.tile_pool(name="expp", bufs=8) as expp, \
             tc.tile_pool(name="avsbp", bufs=3) as avsbp, \
             tc.tile_pool(name="htsp", bufs=4) as htsp, \
             tc.tile_pool(name="recp", bufs=4) as recp, \
             tc.tile_pool(name="stage", bufs=6) as stage, \
             tc.tile_pool(name="scps", bufs=2, space="PSUM") as scps, \
             tc.tile_pool(name="avps", bufs=1, space="PSUM") as avps, \
             tc.tile_pool(name="qkps", bufs=2, space="PSUM") as qkps:

            xts = persist.tile([P, NDC, S], BF16, tag="xts")
            v520 = persist.tile([P, NSC, HPC, DK + 1], BF16, tag="v520")
            hn = persist.tile([P, NPAIR, S], BF16, tag="hn")
            wo_sb = persist.tile([P, NPAIR, D], BF16, tag="wo")
            wv_sb = persist.tile([P, NDC, HPC * DK], BF16, tag="wv")
            id_sb = persist.tile([P, P], BF16, tag="id")

            # ---------------- prologue DMAs ----------------
            wqk_tiles = {}

            def fetch_wqk(pr):
                wqp = wqkp.tile([P, NDC, P], BF16, tag="wqp", name=f"wqp{pr}")
                wkp = wqkp.tile([P, NDC, P], BF16, tag="wkp", name=f"wkp{pr}")
                nc.sync.dma_start(
                    wqp[:], wq_d.ap()[pr].rearrange("p (dc c) -> p dc c", c=P))
                nc.sync.dma_start(
                    wkp[:], wk_d.ap()[pr].rearrange("p (dc c) -> p dc c", c=P))
                wqk_tiles[pr] = (wqp, wkp)

            fetch_wqk(0)
            # xT in 16 chunks; query-block-0 columns first so pair-0 can
            # start; wv before the later chunks (V projection rides early),
            # ident/wo last (needed only from the first norm / pair 3).
            def xts_qtr(qtr):
                for dh in range(NDC // 2):
                    nc.sync.dma_start(
                        xts[:, 2 * dh:2 * dh + 2, qtr * 512:(qtr + 1) * 512],
                        xt_ap[:, 2 * dh:2 * dh + 2, qtr * 512:(qtr + 1) * 512])

            xts_qtr(0)
            xts_qtr(1)
            nc.sync.dma_start(
                wv_sb[:], wv_d.ap().rearrange("(dc p) c -> p dc c", p=P))
            for qtr in range(2, 4):
                xts_qtr(qtr)
            nc.sync.dma_start(id_sb[:], id_d.ap())
            nc.sync.dma_start(
                wo_sb[:], wo_d.ap().rearrange("(ci p) d -> p ci d", p=P))
            nc.vector.memset(v520[:, :, :, DK:DK + 1], 1.0)

            qk_tiles = {0: (qkpool.tile([P, S], BF16, tag="qt", name="qt0"),
                            qkpool.tile([P, S], BF16, tag="kt", name="kt0"))}

            # ~3.4us of dummy matmuls on a zero scratch tile: the PE is
            # waiting on the prologue DMAs anyway, and this lifts it out of
            # the mid p-state before the first real projection group.
            scratch = persist.tile([P, STW], BF16, tag="scratch")
            nc.vector.memset(scratch[:], 0.0)
            for i in range(8):
                wps = qkps.tile([P, STW], F32, tag="qkp", name=f"warm{i}")
                nc.tensor.matmul(wps[:], scratch[:, 0:P], scratch[:],
                                 start=True, stop=True)

            # ---------------- helpers ----------------
            def lp(on):
                return tc.high_priority(offset=-LOWP) if on else nullcontext()

            def qk_group(pr, which, st, sub_sc=False, lowp=False):
                """Project qtp/ktp columns st*512:(st+1)*512 for pair pr."""
                w_sb = wqk_tiles[pr][0 if which == "q" else 1]
                dst = qk_tiles[pr][0 if which == "q" else 1]
                ps = qkps.tile([P, STW], F32, tag="qkp", name=f"{which}{pr}{st}")
                if sub_sc:
                    # key-chunk granular psum + copies (fast prologue start)
                    for scl in range(4):
                        for dc in range(NDC):
                            nc.tensor.matmul(
                                ps[:, scl * P:(scl + 1) * P], w_sb[:, dc, :],
                                xts[:, dc, (st * 4 + scl) * P:(st * 4 + scl + 1) * P],
                                start=(dc == 0), stop=(dc == NDC - 1))
                        nc.vector.tensor_copy(
                            dst[:, (st * 4 + scl) * P:(st * 4 + scl + 1) * P],
                            ps[:, scl * P:(scl + 1) * P])
                else:
                    with lp(lowp):
                        for dc in range(NDC):
                            nc.tensor.matmul(
                                ps[:], w_sb[:, dc, :],
                                xts[:, dc, st * STW:(st + 1) * STW],
                                start=(dc == 0), stop=(dc == NDC - 1))
                        nc.vector.tensor_copy(
                            dst[:, st * STW:(st + 1) * STW], ps[:])

            def v_group(pr, sc, lowp=False):
                """Project V for pair pr's two heads, key chunk sc."""
                ps = qkps.tile([P, STW], F32, tag="qkp", name=f"v{pr}{sc}")
                with lp(lowp):
                    for dc in range(NDC):
                        nc.tensor.matmul(
                            ps[:, 0:P], xts[:, dc, sc * P:(sc + 1) * P],
                            wv_sb[:, dc, pr * P:(pr + 1) * P],
                            start=(dc == 0), stop=(dc == NDC - 1))
                    nc.vector.tensor_copy(
                    v520[:, sc, 2 * pr:2 * pr + 2, 0:DK],
                    ps[:, 0:P].rearrange("p (h v) -> p h v", v=DK))

            def out_group(sc_out, dmh, pool=None, lowp=False):
                if pool is scps:
                    pa = pool.tile([P, 2 * STW], F32, tag="scp",
                                   name=f"o{sc_out}{dmh}")[:, 0:STW]
                else:
                    pa = qkps.tile([P, STW], F32, tag="qkp",
                                   name=f"o{sc_out}{dmh}")[:]
                with lp(lowp):
                    for ci in range(NPAIR):
                        nc.tensor.matmul(
                            pa, hn[:, ci, sc_out * P:(sc_out + 1) * P],
                            wo_sb[:, ci, dmh * 512:(dmh + 1) * 512],
                            start=(ci == 0), stop=(ci == NPAIR - 1))
                    osb = stage.tile([P, STW], F32, tag="ostage", name="osb")
                    nc.vector.tensor_copy(osb[:], pa)
                    nc.sync.dma_start(
                        out_d.ap()[sc_out * P:(sc_out + 1) * P,
                                   dmh * 512:(dmh + 1) * 512],
                        osb[:])

            def scores_exp(pr, qb, sc):
                qtp, ktp = qk_tiles[pr]
                q0 = qb * STW
                scp = scps.tile([P, 2 * STW], F32, tag="scp", name="scp")
                for j in range(2):
                    nc.tensor.matmul(
                        scp[:, j * STW:(j + 1) * STW],
                        ktp[j * DK:(j + 1) * DK, sc * P:(sc + 1) * P],
                        qtp[j * DK:(j + 1) * DK, q0:q0 + STW],
                        start=True, stop=True)
                ex = expp.tile([P, 2 * STW], BF16, tag="ex", name="ex")
                nc.scalar.activation(ex[:], scp[:], EXP, scale=float(SCALE))
                return ex

            av_cur = [None]

            def av_mms(pr, qb, sc, ex):
                if sc == 0:
                    av_cur[0] = [avps.tile([P, NQC * 65], F32, tag=f"av{j}",
                                           name=f"av{j}_{pr}{qb}")
                                 for j in range(2)]
                av = av_cur[0]
                for j in range(2):
                    for qc in range(NQC):
                        nc.tensor.matmul(
                            av[j][:, qc * 65:(qc + 1) * 65],
                            ex[:, j * STW + qc * P:j * STW + (qc + 1) * P],
                            v520[:, sc, 2 * pr + j, :],
                            # start=True zeroes the whole 2KB psum bank, so
                            # only the first of the 4 interleaved qc groups
                            # starts; the rest overwrite-on-first-write via
                            # the bank-wide pending-zero.
                            start=(sc == 0 and qc == 0),
                            stop=(sc == NSC - 1),
                            skip_group_check=True)
                if sc == NSC - 1:
                    if (pr, qb) == (NPAIR - 1, NQB - 1):
                        return av   # epilogue: no next block, read PSUM
                    avsb = avsbp.tile([P, 2, NQC * 65], F32, tag="avsb",
                                      name=f"avsb{pr}{qb}")
                    for j in range(2):
                        nc.vector.tensor_copy(avsb[:, j, :], av[j][:])
                    return avsb
                return None

            def norm_qc(pr, qb, avsb, qc):
                def sl(j, lo, hi):
                    if isinstance(avsb, list):
                        return avsb[j][:, lo:hi]
                    return avsb[:, j, lo:hi]

                """Normalize + transpose one 128-query chunk of both heads.

                The transpose lands in the (between-blocks idle) av0 PSUM
                bank rather than the qkp pool, keeping the qkp rotation
                free for projection / output-projection groups. Emission
                order guarantees the next block's AV matmuls (which re-start
                the bank) come after the hn copies below.
                """
                hts = htsp.tile([P, P], BF16, tag="hts", name=f"h{pr}{qb}{qc}")
                for j in range(2):
                    rec = recp.tile([P, 1], F32, tag="rec", name="rec")
                    nc.vector.reciprocal(
                        rec[:], sl(j, qc * 65 + DK, qc * 65 + DK + 1))
                    nc.vector.tensor_scalar_mul(
                        hts[:, j * DK:(j + 1) * DK],
                        sl(j, qc * 65, qc * 65 + DK), rec[:])
                # the transpose must own its whole PSUM bank: start=True
                # pends the full 2KB zero-region, which silently clobbers
                # any other accumulation group sharing the bank
                tp = qkps.tile([P, STW], F32, tag="qkp", name=f"t{pr}{qb}{qc}")
                tpb = tp[:, 0:P // 2].bitcast(BF16)
                nc.tensor.transpose(tpb, hts[:], id_sb[:])
                nc.vector.tensor_copy(
                    hn[:, pr, (qb * NQC + qc) * P:(qb * NQC + qc + 1) * P],
                    tpb)

            # ---------------- prologue compute (pair 0 heads) --------------
            qk_group(0, "q", 0, sub_sc=True)
            qk_group(0, "k", 0, sub_sc=True)
            for sc in range(4):
                v_group(0, sc)
            for st in range(1, NST):
                qk_group(0, "q", st, lowp=True)
                qk_group(0, "k", st, lowp=True)
            for sc in range(4, NSC):
                v_group(0, sc, lowp=True)

            # ---------------- flat attention pipeline ----------------
            steps = [(pr, qb, sc)
                     for pr in range(NPAIR)
                     for qb in range(NQB)
                     for sc in range(NSC)]

            # Projection filler is released in per-query-block drops sized
            # to each block's PE slack (the scheduler consumes low-priority
            # work eagerly, so one big drop per pair starves the later
            # blocks of that pair). Each drop lands one block before its
            # deadline; pair n's tiles/weights are set up in pair n-1.
            import collections as _c
            drops = _c.defaultdict(list)
            MID = NSC // 2

            def alloc_pair(n):
                def _do():
                    qk_tiles[n] = (
                        qkpool.tile([P, S], BF16, tag="qt", name=f"qt{n}"),
                        qkpool.tile([P, S], BF16, tag="kt", name=f"kt{n}"))
                    fetch_wqk(n)
                return _do

            def qd(n, which, st):
                return lambda: qk_group(n, which, st, lowp=True)

            def vd(n, sc):
                return lambda: v_group(n, sc, lowp=True)

            for n in range(1, NPAIR):
                drops[(n - 1, 1, 0)] += [alloc_pair(n), qd(n, "q", 0)]
                drops[(n - 1, 1, MID)] += [qd(n, "k", 0)]
                drops[(n - 1, 2, 0)] += [qd(n, "q", 1)]
                drops[(n - 1, 2, MID)] += [qd(n, "k", 1)]
                drops[(n - 1, 3, 0)] += [vd(n, sc) for sc in range(4)]
                drops[(n - 1, 3, MID)] += [vd(n, sc) for sc in range(4, 8)]
                drops[(n, 0, 0)] += [qd(n, "k", 2), qd(n, "k", 3)]
                drops[(n, 0, 0)] += [vd(n, sc) for sc in range(8, 12)]
                drops[(n, 0, MID)] += [vd(n, sc) for sc in range(12, NSC)]
                # late Q blocks: inside pair n for n<3, a block earlier for
                # pair 3 whose own span carries the output projections
                if n < NPAIR - 1:
                    drops[(n, 1, 0)] += [qd(n, "q", 2)]
                    drops[(n, 1, MID)] += [qd(n, "q", 3)]
                else:
                    drops[(n - 1, 3, 0)] += [qd(n, "q", 2)]
                    drops[(n - 1, 3, MID)] += [qd(n, "q", 3)]

            exs = {}
            for t in range(len(steps) + 1):
                if t < len(steps):
                    cur = steps[t]
                    if cur[2] in (0, MID):
                        for item in drops.pop(cur, ()):
                            item()
                    exs[cur] = scores_exp(*cur)
                if t > 0:
                    prv = steps[t - 1]
                    avsb = av_mms(*prv, exs.pop(prv))
                    if avsb is not None:
                        ppr, pqb = prv[0], prv[1]
                        for qc in range(NQC):
                            norm_qc(ppr, pqb, avsb, qc)
                            if ppr == NPAIR - 1:
                                so = pqb * NQC + qc
                                if pqb < NQB - 1:
                                    # half now, half at this block's midpoint
                                    # so the scheduler can't burn it all in
                                    # the front of the block
                                    out_group(so, 0, lowp=True)
                                    drops[(ppr, pqb + 1, MID)].append(
                                        lambda so=so: out_group(
                                            so, 1, lowp=True))
                                else:
                                    # epilogue: scores are done, so borrow
                                    # the scp banks for a 4-deep rotation
                                    out_group(so, 0, pool=scps)
                                    out_group(so, 1)

    nc.compile()
    return nc


_NC_CACHE = None


def _get_nc():
    global _NC_CACHE
    if _NC_CACHE is None:
        _NC_CACHE = build_kernel()
    return _NC_CACHE


def kernel(x, Wq, Wk, Wv, Wo):
    import ml_dtypes
    bf16 = ml_dtypes.bfloat16

    x = np.asarray(x, dtype=np.float32)
    Wq = np.asarray(Wq, dtype=np.float32)
    Wk = np.asarray(Wk, dtype=np.float32)
    Wv = np.asarray(Wv, dtype=np.float32)
    Wo = np.asarray(Wo, dtype=np.float32)
    B = x.shape[0]
    ident = np.eye(P, dtype=bf16)

    in_maps = []
    for core in range(8):
        b, g = divmod(core, 2)
        hs = g * HPC
        xt = np.ascontiguousarray(x[b].T).astype(bf16)
        def pack_w(W):
            pairs = []
            for p in range(NPAIR):
                m = np.concatenate([W[hs + 2 * p], W[hs + 2 * p + 1]], axis=1)
                # [D, 128] -> [128 part, NDC*128]: per-partition contiguous
                # so the weight DMA is one 2KB run per partition
                m = m.reshape(NDC, P, P).transpose(1, 0, 2).reshape(P, NDC * P)
                pairs.append(m)
            return np.stack(pairs).astype(bf16)

        wq = pack_w(Wq)
        wk = pack_w(Wk)
        wv = np.concatenate([Wv[hs + h] for h in range(HPC)], axis=1).astype(bf16)
        wo = np.ascontiguousarray(Wo[hs * DK:(hs + HPC) * DK, :]).astype(bf16)
        in_maps.append({"xt": xt, "wq": wq, "wk": wk, "wv": wv, "wo": wo,
                        "ident": ident})

    nc = _get_nc()
    res = run_bass_kernel_spmd(nc, in_maps, core_ids=list(range(8))).results

    out = np.empty((B, S, D), dtype=np.float32)
    for b in range(B):
        out[b] = res[2 * b]["out"] + res[2 * b + 1]["out"]
    return out
